# revision 8
# baseline (speedup 1.0000x reference)
"""Trainium2 Bass kernel for the Dale's-law leaky RNN (nn_Net_20220706030448).

Data-parallel over batch across 8 NeuronCores (B=256 -> 32 per core).
State kept transposed [H-on-partitions, B-free] as a [128, 128] fp32 tile;
recurrent weight held as fp16 stationary tiles (fast weight load), fp32
PSUM accumulation. The input drive (x @ w_in.T + bias) is matmul'd
directly into each step's PSUM tile ahead of time, so the per-step serial
chain is just: 16 rec matmuls -> DVE add -> DVE relu(fp16).
rnn_activity is written fp16 and upcast on the host.
"""

import os

import numpy as np

T, B, I, H, O = 1000, 256, 128, 512, 32
ALPHA = np.float32(20.0 / 100.0)
OMA = np.float32(1.0 - 20.0 / 100.0)
N_CORES = 8
BL = B // N_CORES  # 32 batch per core
TB = 4             # timesteps per block
HB = H // 128      # 4 h-blocks

LAST_RESULTS = None  # test harness reads exec_time_ns from here


def build_bass(t_steps=T):
    import concourse.bacc as bacc
    import concourse.mybir as mybir
    import concourse.tile as tile
    from concourse.masks import make_identity

    f32 = mybir.dt.float32
    f16 = mybir.dt.float16
    Copy = mybir.ActivationFunctionType.Copy
    Max = mybir.AluOpType.max

    nb = t_steps // TB
    assert t_steps % TB == 0

    nc = bacc.Bacc("TRN2", target_bir_lowering=False, debug=False,
                   num_devices=N_CORES)

    x_sl = nc.declare_dram_parameter("x_sl", [t_steps, BL, I], f16, isOutput=False)
    wrec_t = nc.declare_dram_parameter("wrec_t", [H, H], f16, isOutput=False)
    win_t = nc.declare_dram_parameter("win_t", [I, H], f16, isOutput=False)
    fcw_t = nc.declare_dram_parameter("fcw_t", [H, O], f16, isOutput=False)
    bvec16 = nc.declare_dram_parameter("bvec16", [1, H], f16, isOutput=False)
    fcb = nc.declare_dram_parameter("fcb", [O, 1], f32, isOutput=False)
    rnn_T = nc.declare_dram_parameter("rnn_T", [HB, 128, t_steps, BL], f16,
                                      isOutput=True)
    out_T = nc.declare_dram_parameter("out_T", [O, t_steps, BL], f32,
                                      isOutput=True)

    with tile.TileContext(nc) as tc:
        with (
            tc.tile_pool(name="const", bufs=1) as cpool,
            tc.tile_pool(name="xin", bufs=3) as xpool,
            tc.tile_pool(name="xt", bufs=2) as xtpool,
            tc.tile_pool(name="rb16", bufs=2) as rbpool,
            tc.tile_pool(name="ot", bufs=2) as otpool,
            tc.tile_pool(name="ps_rec", bufs=4, space="PSUM") as ps_rec,
            tc.tile_pool(name="ps_x", bufs=2, space="PSUM") as ps_x,
            tc.tile_pool(name="ps_out", bufs=2, space="PSUM") as ps_out,
        ):
            # ---- persistent tiles ----
            wrec_sb = cpool.tile([128, HB * H], f16, tag="wrec")
            for k in range(HB):
                nc.sync.dma_start(wrec_sb[:, k * H:(k + 1) * H],
                                  wrec_t[k * 128:(k + 1) * 128, :])
            win_sb = cpool.tile([128, H], f16, tag="win")
            nc.sync.dma_start(win_sb[:], win_t[:])
            fcw_sb = cpool.tile([128, HB * O], f16, tag="fcw")
            for k in range(HB):
                nc.sync.dma_start(fcw_sb[:, k * O:(k + 1) * O],
                                  fcw_t[k * 128:(k + 1) * 128, :])
            bvec_sb = cpool.tile([1, H], f16, tag="bvec")
            nc.sync.dma_start(bvec_sb[:], bvec16[:])
            fcb_sb = cpool.tile([O, 1], f32, tag="fcb")
            nc.sync.dma_start(fcb_sb[:], fcb[:])

            id_sb = cpool.tile([128, 128], f16, tag="ident")
            make_identity(nc, id_sb[:])
            ones_sb = cpool.tile([1, BL], f16, tag="ones")
            nc.vector.memset(ones_sb[:], 1.0)

            zeros16 = cpool.tile([128, 128], f16, tag="zeros16")
            nc.vector.memset(zeros16[:], 0.0)
            s = cpool.tile([128, 128], f32, tag="state")
            nc.vector.memset(s[:], 0.0)

            # block-level helpers -------------------------------------------
            def load_x(k):
                """DMA x block k and PE-transpose it to xT [i, (t,b)] fp16."""
                t0 = k * TB
                xnat = xpool.tile([128, 128], f16, tag="xnat")
                nc.sync.dma_start(
                    xnat[:], x_sl[t0:t0 + TB].rearrange("t b i -> (t b) i"))
                psx = ps_x.tile([128, 128], f16, tag="psx")
                nc.tensor.transpose(psx[:], xnat[:], id_sb[:])
                xT = xtpool.tile([128, 128], f16, tag="xT")
                nc.scalar.activation(xT[:], psx[:], Copy)
                return xT

            def ph1_slot(xT, psr_tile, t4):
                """Write drive (win @ x_t + bias) into psum tile for step t.

                start=True pending-zeroes the whole 2KB PSUM bank, so only
                the FIRST matmul into the tile may use it; later first-writes
                to other columns see pending-zero and start from 0.
                """
                rhs = xT[:, t4 * BL:(t4 + 1) * BL]
                for m in range(HB):
                    q = slice(m * BL, (m + 1) * BL)
                    nc.tensor.matmul(
                        psr_tile[:, q], win_sb[:, m * 128:(m + 1) * 128], rhs,
                        start=(m == 0), stop=False, skip_group_check=True)
                    nc.tensor.matmul(
                        psr_tile[:, q], bvec_sb[0:1, m * 128:(m + 1) * 128],
                        ones_sb[:], start=False, stop=False,
                        skip_group_check=True)

            # prologue: block 0 drive
            xT_cur = load_x(0)
            psr_tiles = {}
            for t4 in range(TB):
                psr_tiles[t4] = ps_rec.tile([128, 128], f32, tag="psr", name=f"psr_{t4}")
                ph1_slot(xT_cur, psr_tiles[t4], t4)

            prev_rb = None

            for ib in range(nb):
                t0 = ib * TB
                last = ib == nb - 1
                rb = rbpool.tile([128, TB * 128], f16, tag="rb")
                if not last:
                    xT_next = load_x(ib + 1)

                for t4 in range(TB):
                    t = t0 + t4
                    if t == 0:
                        prev = zeros16[:]
                    elif t4 == 0:
                        prev = prev_rb[:, (TB - 1) * 128:TB * 128]
                    else:
                        prev = rb[:, (t4 - 1) * 128:t4 * 128]

                    psr = psr_tiles[t4]
                    for m in range(HB):
                        q = slice(m * BL, (m + 1) * BL)
                        for k in range(HB):
                            nc.tensor.matmul(
                                psr[:, q],
                                wrec_sb[:, k * H + m * 128:k * H + (m + 1) * 128],
                                prev[:, k * BL:(k + 1) * BL],
                                start=False, stop=(k == HB - 1),
                                skip_group_check=True)

                    # state update + relu (the serial chain)
                    nc.vector.tensor_scalar_mul(s[:], s[:], float(OMA))
                    nc.vector.tensor_add(s[:], s[:], psr[:])
                    nc.vector.tensor_scalar(
                        rb[:, t4 * 128:(t4 + 1) * 128], s[:], 0.0, None, Max)

                    # refill this psum slot with the next block's drive
                    # (fills the chain bubble after this step's rec matmuls)
                    if not last:
                        psr_tiles[t4] = ps_rec.tile([128, 128], f32, tag="psr", name=f"psr_{t4}")
                        ph1_slot(xT_next, psr_tiles[t4], t4)

                # output projection for the block: out_T[o, t, b]
                rb_v = rb[:].rearrange("p (t k b) -> p t k b", t=TB, k=HB, b=BL)
                pso = ps_out.tile([O, TB * BL], f32, tag="pso")
                for k in range(HB):
                    nc.tensor.matmul(
                        pso[:], fcw_sb[:, k * O:(k + 1) * O], rb_v[:, :, k, :],
                        start=(k == 0), stop=(k == HB - 1))
                oT = otpool.tile([O, TB * BL], f32, tag="oT")
                nc.vector.tensor_scalar_add(oT[:], pso[:], fcb_sb[:, 0:1])
                nc.sync.dma_start(
                    out_T[:, t0:t0 + TB, :].rearrange("o t b -> o (t b)"), oT[:])

                # write fp16 relu block to rnn_T[k, p, t0:t0+4, :]
                for k in range(HB):
                    nc.sync.dma_start(rnn_T[k, :, t0:t0 + TB, :], rb_v[:, :, k, :])

                if not last:
                    xT_cur = xT_next
                prev_rb = rb

    nc.compile()
    return nc


_BUILT = {}


def _get_nc(t_steps):
    if t_steps not in _BUILT:
        _BUILT[t_steps] = build_bass(t_steps)
    return _BUILT[t_steps]


def host_prep(w_in, b_in, w_h, b_h, dale, sparse, fc_w, fc_b):
    w_eff = np.maximum(w_h, 0.0) * dale[None, :] * sparse          # [H, H]
    wrec_t = (ALPHA * w_eff).T.astype(np.float16).copy()           # [H, H]
    win_t = (ALPHA * w_in).T.astype(np.float16).copy()             # [I, H]
    fcw_t = fc_w.T.astype(np.float16).copy()                       # [H, O]
    bvec16 = (ALPHA * (b_in + b_h)).astype(np.float16).reshape(1, H).copy()
    fcb = fc_b.astype(np.float32).reshape(O, 1).copy()
    return wrec_t, win_t, fcw_t, bvec16, fcb


def kernel(x, w_in, b_in, w_h, b_h, dale, sparse, fc_w, fc_b):
    from concourse.bass_utils import run_bass_kernel_spmd

    global LAST_RESULTS
    x = np.asarray(x, dtype=np.float32)
    wrec_t, win_t, fcw_t, bvec16, fcb = host_prep(
        np.asarray(w_in, np.float32), np.asarray(b_in, np.float32),
        np.asarray(w_h, np.float32), np.asarray(b_h, np.float32),
        np.asarray(dale, np.float32), np.asarray(sparse, np.float32),
        np.asarray(fc_w, np.float32), np.asarray(fc_b, np.float32))

    t_steps = x.shape[0]
    nc = _get_nc(t_steps)

    x16 = x.astype(np.float16)
    in_maps = []
    for c in range(N_CORES):
        in_maps.append({
            "x_sl": np.ascontiguousarray(x16[:, c * BL:(c + 1) * BL, :]),
            "wrec_t": wrec_t, "win_t": win_t, "fcw_t": fcw_t,
            "bvec16": bvec16, "fcb": fcb,
        })

    trace = bool(os.environ.get("BASS_TRACE"))
    res = run_bass_kernel_spmd(nc, in_maps, list(range(N_CORES)), trace=trace,
                               tmpdir=os.environ.get("BASS_TRACE_DIR"))
    LAST_RESULTS = res

    out = np.empty((t_steps, B, O), dtype=np.float32)
    rnn = np.empty((t_steps, B, H), dtype=np.float32)
    for c in range(N_CORES):
        r = res.results[c]
        bs = slice(c * BL, (c + 1) * BL)
        # rnn_T[k, p, t, b] (fp16) -> [t, b, k*128+p] fp32
        rnn[:, bs, :] = np.transpose(
            r["rnn_T"].astype(np.float32), (2, 3, 0, 1)).reshape(t_steps, BL, H)
        out[:, bs, :] = np.transpose(r["out_T"], (1, 2, 0))
    return out, rnn


# revision 14
# speedup vs baseline: 1.3884x; 1.3884x over previous
"""Trainium2 Bass kernel for the Dale's-law leaky RNN (nn_Net_20220706030448).

Data-parallel over batch across 8 NeuronCores (B=256 -> 32 per core).
State kept transposed [H-on-partitions, B-free] as a [128, 128] fp32 tile;
recurrent weight held as fp16 stationary tiles (fast weight load), fp32
PSUM accumulation. The input drive (x @ w_in.T + bias) is matmul'd
directly into each step's PSUM tile ahead of time, so the per-step serial
chain is just: 16 rec matmuls -> DVE add -> DVE relu(fp16).
rnn_activity is written fp16 and upcast on the host.
"""

import os

import numpy as np

T, B, I, H, O = 1000, 256, 128, 512, 32
ALPHA = np.float32(20.0 / 100.0)
OMA = np.float32(1.0 - 20.0 / 100.0)
N_CORES = 8
BL = B // N_CORES  # 32 batch per core
TB = 4             # timesteps per block
HB = H // 128      # 4 h-blocks

LAST_RESULTS = None  # test harness reads exec_time_ns from here


def build_bass(t_steps=T):
    import concourse.bacc as bacc
    import concourse.mybir as mybir
    import concourse.tile as tile
    from concourse.masks import make_identity

    f32 = mybir.dt.float32
    f16 = mybir.dt.float16
    Copy = mybir.ActivationFunctionType.Copy
    Max = mybir.AluOpType.max

    nb = t_steps // TB
    assert t_steps % TB == 0

    nc = bacc.Bacc("TRN2", target_bir_lowering=False, debug=False,
                   num_devices=N_CORES)

    x_sl = nc.declare_dram_parameter("x_sl", [t_steps, BL, I], f16, isOutput=False)
    wrec_t = nc.declare_dram_parameter("wrec_t", [H, H], f16, isOutput=False)
    win_t = nc.declare_dram_parameter("win_t", [I, H], f16, isOutput=False)
    fcw_t = nc.declare_dram_parameter("fcw_t", [H, O], f16, isOutput=False)
    bvec4 = nc.declare_dram_parameter("bvec4", [HB, 128], f16, isOutput=False)
    mask4 = nc.declare_dram_parameter("mask4", [HB, 128], f16, isOutput=False)
    fcb = nc.declare_dram_parameter("fcb", [O, 1], f32, isOutput=False)
    rnn_T = nc.declare_dram_parameter("rnn_T", [HB, 128, t_steps, BL], f16,
                                      isOutput=True)
    out_T = nc.declare_dram_parameter("out_T", [O, t_steps, BL], f32,
                                      isOutput=True)

    with tile.TileContext(nc) as tc:
        with (
            tc.tile_pool(name="const", bufs=1) as cpool,
            tc.tile_pool(name="xin", bufs=3) as xpool,
            tc.tile_pool(name="xt", bufs=2) as xtpool,
            tc.tile_pool(name="rb16", bufs=2) as rbpool,
            tc.tile_pool(name="ot", bufs=2) as otpool,
            tc.tile_pool(name="ps_rec", bufs=4, space="PSUM") as ps_rec,
            tc.tile_pool(name="ps_x", bufs=2, space="PSUM") as ps_x,
            tc.tile_pool(name="ps_out", bufs=2, space="PSUM") as ps_out,
        ):
            # ---- persistent tiles ----
            wrec_sb = cpool.tile([128, HB * H], f16, tag="wrec")
            for k in range(HB):
                nc.sync.dma_start(wrec_sb[:, k * H:(k + 1) * H],
                                  wrec_t[k * 128:(k + 1) * 128, :])
            win_sb = cpool.tile([128, H], f16, tag="win")
            nc.sync.dma_start(win_sb[:], win_t[:])
            fcw_sb = cpool.tile([128, HB * O], f16, tag="fcw")
            for k in range(HB):
                nc.sync.dma_start(fcw_sb[:, k * O:(k + 1) * O],
                                  fcw_t[k * 128:(k + 1) * 128, :])
            bvec_sb = cpool.tile([HB, 128], f16, tag="bvec")
            nc.sync.dma_start(bvec_sb[:], bvec4[:])
            mask_sb = cpool.tile([HB, 128], f16, tag="mask")
            nc.sync.dma_start(mask_sb[:], mask4[:])
            fcb_sb = cpool.tile([O, 1], f32, tag="fcb")
            nc.sync.dma_start(fcb_sb[:], fcb[:])

            id_sb = cpool.tile([128, 128], f16, tag="ident")
            make_identity(nc, id_sb[:])

            zeros16 = cpool.tile([128, 128], f16, tag="zeros16")
            nc.vector.memset(zeros16[:], 0.0)
            s = cpool.tile([128, 128], f32, tag="state")
            nc.vector.memset(s[:], 0.0)

            # block-level helpers -------------------------------------------
            def load_x(k):
                """DMA x block k and PE-transpose it to xT [i, (t,b)] fp16."""
                t0 = k * TB
                xnat = xpool.tile([128, 128], f16, tag="xnat")
                nc.sync.dma_start(
                    xnat[:], x_sl[t0:t0 + TB].rearrange("t b i -> (t b) i"))
                psx = ps_x.tile([128, 128], f16, tag="psx")
                nc.tensor.transpose(psx[:], xnat[:], id_sb[:])
                xT = xtpool.tile([128, 128], f16, tag="xT")
                nc.scalar.activation(xT[:], psx[:], Copy)
                return xT

            def ph1_slot(xT, psr_tile, t4):
                """Write drive (win @ x_t) into the psum tile for step t.

                start=True pending-zeroes the whole 2KB PSUM bank, so only
                the FIRST matmul into the tile may use it; later first-writes
                to other columns see pending-zero and start from 0.
                """
                rhs = xT[:, t4 * BL:(t4 + 1) * BL]
                for m in range(HB):
                    q = slice(m * BL, (m + 1) * BL)
                    nc.tensor.matmul(
                        psr_tile[:, q], win_sb[:, m * 128:(m + 1) * 128], rhs,
                        start=(m == 0), stop=False, skip_group_check=True)

            def bias_slot(psr_tile):
                """Accumulate bias[m*128+p] into col (m,b) as a K=4 matmul:
                out[p, c] = sum_j bvec4[j, p] * (c//32 == j)."""
                nc.tensor.matmul(
                    psr_tile[:], bvec_sb[:], mask_sb[:],
                    start=False, stop=False, skip_group_check=True)

            # prologue: block 0 drive
            xT_cur = load_x(0)
            psr_tiles = {}
            for t4 in range(TB):
                psr_tiles[t4] = ps_rec.tile([128, 128], f32, tag="psr", name=f"psr_{t4}")
                ph1_slot(xT_cur, psr_tiles[t4], t4)
            for t4 in range(TB):
                bias_slot(psr_tiles[t4])

            prev_rb = None

            for ib in range(nb):
                t0 = ib * TB
                last = ib == nb - 1
                rb = rbpool.tile([128, TB * 128], f16, tag="rb")
                if not last:
                    xT_next = load_x(ib + 1)
                new_tiles = {}

                for t4 in range(TB):
                    t = t0 + t4
                    if t == 0:
                        prev = zeros16[:]
                    elif t4 == 0:
                        prev = prev_rb[:, (TB - 1) * 128:TB * 128]
                    else:
                        prev = rb[:, (t4 - 1) * 128:t4 * 128]

                    psr = psr_tiles[t4]
                    for m in range(HB):
                        q = slice(m * BL, (m + 1) * BL)
                        for k in range(HB):
                            nc.tensor.matmul(
                                psr[:, q],
                                wrec_sb[:, k * H + m * 128:k * H + (m + 1) * 128],
                                prev[:, k * BL:(k + 1) * BL],
                                start=False, stop=(k == HB - 1),
                                skip_group_check=True)

                    # refill the PREVIOUS step's psum slot with the next
                    # block's drive (lands in this step's chain bubble, after
                    # this step's rec matmuls in PE program order)
                    if not last and t4 >= 1:
                        j = t4 - 1
                        new_tiles[j] = ps_rec.tile([128, 128], f32, tag="psr",
                                                   name=f"psr_{j}")
                        ph1_slot(xT_next, new_tiles[j], j)

                    # state update + relu (the serial chain)
                    nc.vector.tensor_scalar_mul(s[:], s[:], float(OMA))
                    nc.vector.tensor_add(s[:], s[:], psr[:])
                    nc.vector.tensor_scalar(
                        rb[:, t4 * 128:(t4 + 1) * 128], s[:], 0.0, None, Max)

                if not last:
                    new_tiles[TB - 1] = ps_rec.tile([128, 128], f32, tag="psr",
                                                    name=f"psr_{TB - 1}")
                    ph1_slot(xT_next, new_tiles[TB - 1], TB - 1)
                    for t4 in range(TB):
                        bias_slot(new_tiles[t4])
                    psr_tiles = new_tiles

                # output projection for the block: out_T[o, t, b]
                rb_v = rb[:].rearrange("p (t k b) -> p t k b", t=TB, k=HB, b=BL)
                pso = ps_out.tile([O, TB * BL], f32, tag="pso")
                for k in range(HB):
                    nc.tensor.matmul(
                        pso[:], fcw_sb[:, k * O:(k + 1) * O], rb_v[:, :, k, :],
                        start=(k == 0), stop=(k == HB - 1))
                oT = otpool.tile([O, TB * BL], f32, tag="oT")
                nc.vector.tensor_scalar_add(oT[:], pso[:], fcb_sb[:, 0:1])
                nc.sync.dma_start(
                    out_T[:, t0:t0 + TB, :].rearrange("o t b -> o (t b)"), oT[:])

                # write fp16 relu block to rnn_T[k, p, t0:t0+4, :]
                for k in range(HB):
                    nc.sync.dma_start(rnn_T[k, :, t0:t0 + TB, :], rb_v[:, :, k, :])

                if not last:
                    xT_cur = xT_next
                prev_rb = rb

    nc.compile()
    return nc


_BUILT = {}


def _get_nc(t_steps):
    if t_steps not in _BUILT:
        _BUILT[t_steps] = build_bass(t_steps)
    return _BUILT[t_steps]


def host_prep(w_in, b_in, w_h, b_h, dale, sparse, fc_w, fc_b):
    w_eff = np.maximum(w_h, 0.0) * dale[None, :] * sparse          # [H, H]
    wrec_t = (ALPHA * w_eff).T.astype(np.float16).copy()           # [H, H]
    win_t = (ALPHA * w_in).T.astype(np.float16).copy()             # [I, H]
    fcw_t = fc_w.T.astype(np.float16).copy()                       # [H, O]
    bvec4 = (ALPHA * (b_in + b_h)).astype(np.float16).reshape(HB, 128).copy()
    mask4 = (np.arange(TB * BL)[None, :] // BL ==
             np.arange(HB)[:, None]).astype(np.float16)            # [HB, 128]
    fcb = fc_b.astype(np.float32).reshape(O, 1).copy()
    return wrec_t, win_t, fcw_t, bvec4, mask4, fcb


def kernel(x, w_in, b_in, w_h, b_h, dale, sparse, fc_w, fc_b):
    from concourse.bass_utils import run_bass_kernel_spmd

    global LAST_RESULTS
    x = np.asarray(x, dtype=np.float32)
    wrec_t, win_t, fcw_t, bvec4, mask4, fcb = host_prep(
        np.asarray(w_in, np.float32), np.asarray(b_in, np.float32),
        np.asarray(w_h, np.float32), np.asarray(b_h, np.float32),
        np.asarray(dale, np.float32), np.asarray(sparse, np.float32),
        np.asarray(fc_w, np.float32), np.asarray(fc_b, np.float32))

    t_steps = x.shape[0]
    nc = _get_nc(t_steps)

    x16 = x.astype(np.float16)
    in_maps = []
    for c in range(N_CORES):
        in_maps.append({
            "x_sl": np.ascontiguousarray(x16[:, c * BL:(c + 1) * BL, :]),
            "wrec_t": wrec_t, "win_t": win_t, "fcw_t": fcw_t,
            "bvec4": bvec4, "mask4": mask4, "fcb": fcb,
        })

    trace = bool(os.environ.get("BASS_TRACE"))
    res = run_bass_kernel_spmd(nc, in_maps, list(range(N_CORES)), trace=trace,
                               tmpdir=os.environ.get("BASS_TRACE_DIR"))
    LAST_RESULTS = res

    out = np.empty((t_steps, B, O), dtype=np.float32)
    rnn = np.empty((t_steps, B, H), dtype=np.float32)
    for c in range(N_CORES):
        r = res.results[c]
        bs = slice(c * BL, (c + 1) * BL)
        # rnn_T[k, p, t, b] (fp16) -> [t, b, k*128+p] fp32
        rnn[:, bs, :] = np.transpose(
            r["rnn_T"].astype(np.float32), (2, 3, 0, 1)).reshape(t_steps, BL, H)
        out[:, bs, :] = np.transpose(r["out_T"], (1, 2, 0))
    return out, rnn


# revision 15
# speedup vs baseline: 1.6842x; 1.2130x over previous
"""Trainium2 Bass kernel for the Dale's-law leaky RNN (nn_Net_20220706030448).

Data-parallel over batch across 8 NeuronCores (B=256 -> 32 per core).
State kept transposed [H-on-partitions, B-free] as a [128, 128] fp32 tile;
all weights fp16 (fast weight load), fp32 PSUM accumulation.

Per 4-step block the state is kept pre-scaled (s~_j = s_{t0+j} * w^-(j+1)),
with the scale folded into per-phase weight copies, so the per-step serial
chain is just: 16 rec matmuls -> DVE add -> DVE relu(fp16). One s *= w^4
renormalization per block. The input drive (x @ w_in.T + bias) and the
output projection run as full-array K=128/M=128 matmuls interleaved into
the chain bubbles. rnn_activity is written fp16 (phase-scaled) and fixed
up on the host.
"""

import os

import numpy as np

T, B, I, H, O = 1000, 256, 128, 512, 32
ALPHA = np.float32(20.0 / 100.0)
OMA = np.float32(1.0 - 20.0 / 100.0)
N_CORES = 8
BL = B // N_CORES  # 32 batch per core
TB = 4             # timesteps per block
HB = H // 128      # 4 h-blocks

LAST_RESULTS = None  # test harness reads exec_time_ns from here


def build_bass(t_steps=T):
    import concourse.bacc as bacc
    import concourse.mybir as mybir
    import concourse.tile as tile
    from concourse.masks import make_identity

    f32 = mybir.dt.float32
    f16 = mybir.dt.float16
    Copy = mybir.ActivationFunctionType.Copy
    Max = mybir.AluOpType.max

    nb = t_steps // TB
    assert t_steps % TB == 0

    nc = bacc.Bacc("TRN2", target_bir_lowering=False, debug=False,
                   num_devices=N_CORES)

    x_sl = nc.declare_dram_parameter("x_sl", [t_steps, BL, I], f16, isOutput=False)
    # host-prelayouted fp16 weight planes (see host_prep)
    wrec_t = nc.declare_dram_parameter("wrec_t", [128, 2 * HB * H], f16, isOutput=False)
    win_t = nc.declare_dram_parameter("win_t", [128, TB * H], f16, isOutput=False)
    fcw_t = nc.declare_dram_parameter("fcw_t", [128, TB * HB * 128], f16, isOutput=False)
    bvec_t = nc.declare_dram_parameter("bvec_t", [128, 128], f16, isOutput=False)
    maskd_t = nc.declare_dram_parameter("maskd_t", [128, TB * 128], f16, isOutput=False)
    fcb = nc.declare_dram_parameter("fcb", [O, 1], f32, isOutput=False)
    rnn_T = nc.declare_dram_parameter("rnn_T", [HB, 128, t_steps, BL], f16,
                                      isOutput=True)
    out_T = nc.declare_dram_parameter("out_T", [O, t_steps, BL], f32,
                                      isOutput=True)

    with tile.TileContext(nc) as tc:
        with (
            tc.tile_pool(name="const", bufs=1) as cpool,
            tc.tile_pool(name="xin", bufs=3) as xpool,
            tc.tile_pool(name="xt", bufs=2) as xtpool,
            tc.tile_pool(name="rb16", bufs=2) as rbpool,
            tc.tile_pool(name="ot", bufs=2) as otpool,
            tc.tile_pool(name="ps_rec", bufs=4, space="PSUM") as ps_rec,
            tc.tile_pool(name="ps_x", bufs=2, space="PSUM") as ps_x,
            tc.tile_pool(name="ps_out", bufs=2, space="PSUM") as ps_out,
        ):
            # ---- persistent tiles ----
            wrec_sb = cpool.tile([128, 2 * HB * H], f16, tag="wrec")
            nc.sync.dma_start(wrec_sb[:], wrec_t[:])
            win_sb = cpool.tile([128, TB * H], f16, tag="win")
            nc.sync.dma_start(win_sb[:], win_t[:])
            fcw_sb = cpool.tile([128, TB * HB * 128], f16, tag="fcw")
            nc.sync.dma_start(fcw_sb[:], fcw_t[:])
            bvec_sb = cpool.tile([128, 128], f16, tag="bvec")
            nc.sync.dma_start(bvec_sb[:], bvec_t[:])
            maskd_sb = cpool.tile([128, TB * 128], f16, tag="maskd")
            nc.sync.dma_start(maskd_sb[:], maskd_t[:])
            fcb_sb = cpool.tile([O, 1], f32, tag="fcb")
            nc.sync.dma_start(fcb_sb[:], fcb[:])

            id_sb = cpool.tile([128, 128], f16, tag="ident")
            make_identity(nc, id_sb[:])

            zeros16 = cpool.tile([128, 128], f16, tag="zeros16")
            nc.vector.memset(zeros16[:], 0.0)
            s = cpool.tile([128, 128], f32, tag="state")
            nc.vector.memset(s[:], 0.0)

            # block-level helpers -------------------------------------------
            def load_x(k):
                """DMA x block k and PE-transpose it to xT [i, (t,b)] fp16."""
                t0 = k * TB
                xnat = xpool.tile([128, 128], f16, tag="xnat")
                nc.sync.dma_start(
                    xnat[:], x_sl[t0:t0 + TB].rearrange("t b i -> (t b) i"))
                psx = ps_x.tile([128, 128], f16, tag="psx")
                nc.tensor.transpose(psx[:], xnat[:], id_sb[:])
                xT = xtpool.tile([128, 128], f16, tag="xT")
                nc.scalar.activation(xT[:], psx[:], Copy)
                return xT

            def ph1_slot(xT, psr_tile, t4):
                """Drive (phase-scaled win @ x_t + bias) into step t's psum.

                start=True pending-zeroes the whole 2KB PSUM bank, so only
                the FIRST matmul into the tile may use it.
                """
                rhs = xT[:, t4 * BL:(t4 + 1) * BL]
                for m in range(HB):
                    q = slice(m * BL, (m + 1) * BL)
                    nc.tensor.matmul(
                        psr_tile[:, q],
                        win_sb[:, t4 * H + m * 128:t4 * H + (m + 1) * 128],
                        rhs, start=(m == 0), stop=False, skip_group_check=True)
                # bias: out[p, (m,b)] = sum_k bvec_rep[k, p] * maskd[k, (m,b)]
                nc.tensor.matmul(
                    psr_tile[:], bvec_sb[:],
                    maskd_sb[:, t4 * 128:(t4 + 1) * 128],
                    start=False, stop=False, skip_group_check=True)

            def ph3_step(rb, pso, t4):
                """Accumulate fc_w @ relu_t into the block's out psum."""
                for k in range(HB):
                    nc.tensor.matmul(
                        pso[:, t4 * BL:(t4 + 1) * BL],
                        fcw_sb[:, (t4 * HB + k) * 128:(t4 * HB + k + 1) * 128],
                        rb[:, t4 * 128 + k * BL:t4 * 128 + (k + 1) * BL],
                        start=(t4 == 0 and k == 0), stop=False,
                        skip_group_check=True)

            # prologue: block 0 drive
            xT_cur = load_x(0)
            psr_tiles = {}
            for t4 in range(TB):
                psr_tiles[t4] = ps_rec.tile([128, 128], f32, tag="psr",
                                            name=f"psr_{t4}")
                ph1_slot(xT_cur, psr_tiles[t4], t4)

            prev_rb = None

            for ib in range(nb):
                t0 = ib * TB
                last = ib == nb - 1
                rb = rbpool.tile([128, TB * 128], f16, tag="rb")
                if not last:
                    xT_next = load_x(ib + 1)
                new_tiles = {}
                pso = ps_out.tile([128, TB * BL], f32, tag="pso")

                for t4 in range(TB):
                    t = t0 + t4
                    if t == 0:
                        prev = zeros16[:]
                    elif t4 == 0:
                        prev = prev_rb[:, (TB - 1) * 128:TB * 128]
                    else:
                        prev = rb[:, (t4 - 1) * 128:t4 * 128]

                    # phase 0 consumes prev scaled w^-4 (pre-renorm handled
                    # by using the alpha*w^3 copy); phases 1-3 use alpha/w
                    wbase = 0 if t4 == 0 else HB * H
                    psr = psr_tiles[t4]
                    for m in range(HB):
                        q = slice(m * BL, (m + 1) * BL)
                        for k in range(HB):
                            nc.tensor.matmul(
                                psr[:, q],
                                wrec_sb[:, wbase + k * H + m * 128:
                                        wbase + k * H + (m + 1) * 128],
                                prev[:, k * BL:(k + 1) * BL],
                                start=False, stop=(k == HB - 1),
                                skip_group_check=True)

                    # bubble fill: previous step's out-projection + refill
                    if t4 >= 1:
                        ph3_step(rb, pso, t4 - 1)
                        if not last:
                            j = t4 - 1
                            new_tiles[j] = ps_rec.tile([128, 128], f32,
                                                       tag="psr", name=f"psr_{j}")
                            ph1_slot(xT_next, new_tiles[j], j)

                    # the serial chain: s~ += drive ; rb = relu(s~) fp16
                    nc.vector.tensor_add(s[:], s[:], psr[:])
                    nc.vector.tensor_scalar(
                        rb[:, t4 * 128:(t4 + 1) * 128], s[:], 0.0, None, Max)

                # block epilogue ------------------------------------------
                ph3_step(rb, pso, TB - 1)
                if not last:
                    j = TB - 1
                    new_tiles[j] = ps_rec.tile([128, 128], f32, tag="psr",
                                               name=f"psr_{j}")
                    ph1_slot(xT_next, new_tiles[j], j)
                    psr_tiles = new_tiles
                    # renormalize the pre-scaled state: s = s~ * w^4
                    nc.vector.tensor_scalar_mul(s[:], s[:], float(OMA) ** TB)

                oT = otpool.tile([O, TB * BL], f32, tag="oT")
                nc.vector.tensor_scalar_add(oT[:], pso[0:O, :], fcb_sb[:, 0:1])
                nc.sync.dma_start(
                    out_T[:, t0:t0 + TB, :].rearrange("o t b -> o (t b)"), oT[:])

                rb_v = rb[:].rearrange("p (t k b) -> p t k b", t=TB, k=HB, b=BL)
                for k in range(HB):
                    nc.sync.dma_start(rnn_T[k, :, t0:t0 + TB, :], rb_v[:, :, k, :])

                if not last:
                    xT_cur = xT_next
                prev_rb = rb

    nc.compile()
    return nc


_BUILT = {}


def _get_nc(t_steps):
    if t_steps not in _BUILT:
        _BUILT[t_steps] = build_bass(t_steps)
    return _BUILT[t_steps]


def host_prep(w_in, b_in, w_h, b_h, dale, sparse, fc_w, fc_b):
    om = float(OMA)
    w_eff = np.maximum(w_h, 0.0) * dale[None, :] * sparse          # [H, H]

    # wrec planes: [p, c*2048 + k*512 + m*128 + q] = scale_c * w_eff[m*128+q, k*128+p]
    wrec = np.empty((128, 2 * HB * H), np.float16)
    weT = w_eff.T  # [h(k), h'(m)]
    for ci, scale in enumerate([ALPHA * om ** (TB - 1), ALPHA / om]):
        for k in range(HB):
            blk = (scale * weT[k * 128:(k + 1) * 128, :]).astype(np.float16)
            wrec[:, ci * HB * H + k * H:(ci * HB + k + 1) * H] = blk

    # win planes: [i, j*512 + h'] = alpha * w^-(j+1) * w_in[h', i]
    win = np.empty((128, TB * H), np.float16)
    wiT = w_in.T  # [i, h']
    for j in range(TB):
        win[:, j * H:(j + 1) * H] = (ALPHA * om ** (-(j + 1)) * wiT
                                     ).astype(np.float16)

    # fcw planes (padded M=128): [p, (j*HB+k)*128 + c] = w^(j+1)*fc_w[c, k*128+p]
    # for c < O, else 0
    fcw = np.zeros((128, TB * HB * 128), np.float16)
    fwT = fc_w.T  # [h, o]
    for j in range(TB):
        for k in range(HB):
            col = (j * HB + k) * 128
            fcw[:, col:col + O] = (om ** (j + 1) *
                                   fwT[k * 128:(k + 1) * 128, :]).astype(np.float16)

    bv = (b_in + b_h).astype(np.float32)                           # [H]
    bvec_rep = np.empty((128, 128), np.float16)
    for k in range(128):
        bvec_rep[k, :] = bv[(k // BL) * 128:(k // BL + 1) * 128]

    # maskd: [k, j*128 + c] = alpha * w^-(j+1) / 32 * (c//32 == k//32)
    maskd = np.zeros((128, TB * 128), np.float16)
    base = (np.arange(128)[:, None] // BL ==
            np.arange(128)[None, :] // BL).astype(np.float32) / BL
    for j in range(TB):
        maskd[:, j * 128:(j + 1) * 128] = (ALPHA * om ** (-(j + 1)) * base
                                           ).astype(np.float16)

    fcb = fc_b.astype(np.float32).reshape(O, 1).copy()
    return wrec, win, fcw, bvec_rep, maskd, fcb


def kernel(x, w_in, b_in, w_h, b_h, dale, sparse, fc_w, fc_b):
    from concourse.bass_utils import run_bass_kernel_spmd

    global LAST_RESULTS
    x = np.asarray(x, dtype=np.float32)
    wrec, win, fcw, bvec_rep, maskd, fcb = host_prep(
        np.asarray(w_in, np.float32), np.asarray(b_in, np.float32),
        np.asarray(w_h, np.float32), np.asarray(b_h, np.float32),
        np.asarray(dale, np.float32), np.asarray(sparse, np.float32),
        np.asarray(fc_w, np.float32), np.asarray(fc_b, np.float32))

    t_steps = x.shape[0]
    nc = _get_nc(t_steps)

    x16 = x.astype(np.float16)
    in_maps = []
    for c in range(N_CORES):
        in_maps.append({
            "x_sl": np.ascontiguousarray(x16[:, c * BL:(c + 1) * BL, :]),
            "wrec_t": wrec, "win_t": win, "fcw_t": fcw,
            "bvec_t": bvec_rep, "maskd_t": maskd, "fcb": fcb,
        })

    trace = bool(os.environ.get("BASS_TRACE"))
    res = run_bass_kernel_spmd(nc, in_maps, list(range(N_CORES)), trace=trace,
                               tmpdir=os.environ.get("BASS_TRACE_DIR"))
    LAST_RESULTS = res

    # raw rnn_T holds relu(s~_j) = w^-(j+1) * relu(s_t); undo the phase scale
    scales = (float(OMA) ** ((np.arange(t_steps) % TB) + 1)).astype(np.float32)
    out = np.empty((t_steps, B, O), dtype=np.float32)
    rnn = np.empty((t_steps, B, H), dtype=np.float32)
    for c in range(N_CORES):
        r = res.results[c]
        bs = slice(c * BL, (c + 1) * BL)
        arr = np.transpose(r["rnn_T"].astype(np.float32),
                           (2, 3, 0, 1)).reshape(t_steps, BL, H)
        arr *= scales[:, None, None]
        rnn[:, bs, :] = arr
        out[:, bs, :] = np.transpose(r["out_T"], (1, 2, 0))
    return out, rnn
